# revision 52
# baseline (speedup 1.0000x reference)
"""Causal self-attention Trainium2 Bass kernel (fp16 SBUF-resident rewrite).

Problem: B=4, T=2048, C=2048, H=16 heads, D=128 head dim, fp32 in/out.
  qkv = x @ w_qkv ; causal softmax(q k^T / sqrt(D)) v ; out = av @ w_proj

Sharding (8 NeuronCores): DP=4 over batch x TP=2 over head groups
(Megatron-style: w_qkv columns / w_proj rows split by heads). Core
c handles batch b=c//2, heads g*8..g*8+8 with g=c%2. Each core emits a
partial [T, C] f32 output; host sums the TP pairs.

Key design points (in rough order of discovery):
  * All operands stored fp16 (inputs cast host-side); matmuls run fp16
    at the same 1 cycle/row PE rate as f32r but with half the SBUF/DMA
    traffic, so q/k/v/av stay SBUF-resident end to end (no DRAM round
    trip between phases).
  * Exp runs in [128,1024] two-PSUM-bank mega tiles (halves the ACT
    per-instruction access overhead); a diagonal pair left-aligns its
    second block so the pair is one contiguous exp with no garbage.
  * Phase 2 is one globally software-pipelined stream of block-pairs:
    scores run LOOK=7 pairs ahead of AV, and the softmax close-out
    chain (den ones-matmuls -> DVE reciprocal -> gpsimd broadcast) is
    emitted DEN_DELAY-1 pairs after a group's last score pair, i.e.
    several pairs BEFORE its last AV matmuls, so the final rescale
    multiply never stalls the PE or convoys the DVE FIFO.
  * Sum accumulation is [128,1024]-wide: clean pairs fold in with ONE
    2x-mode DVE add, pair 0 initializes via a 4x-mode copy, and sums
    are emitted lazily (one pair late) so the exp-gating mask adds
    never queue behind them. si0 skips sumacc entirely (its den reads
    both et tiles directly). The den ones-matmul must use a 1-column
    stationary: a 128-column (pre-broadcast) ones matmul trips the P0
    power downclock (PE 2.4 -> 2.0 GHz for the entire kernel).
  * gpsimd runs EXACTLY ONE compute op type (partition_broadcast):
    mixing op types reloads the Q7 library at ~6us per switch. A dummy
    broadcast in phase 1 preloads the library off the critical path.
  * Blocks are woven (si3,si1)+(si2,si0) with the short group placed in
    the first ~55% so consecutive close-out chains never collide.
  * ~24 dummy matmuls on a memset tile warm the HAM clock gate (4/8 ->
    8/8 at 2.4 GHz) during the DMA-ramp dead time at kernel start, and
    the startup-critical x/w chunks ride one queue in first-use order.
  * Phase 3 is merged into the phase-2 pool scope (PSUM reuses tag
    "sc") with tch emitted in avts-readiness order, so projection
    matmuls start while the last head's close-outs drain and the PE
    never idles long enough for a HAM rethrottle at the seam.
  * Phase-3 partials are written f16 (summed in f32 on host); wp loads
    are drip-fed on the idle sync queue during phase 2.
"""

import math
import os
import sys
from collections import deque

import numpy as np

for _p in ("/opt/trn_rl_repo",):
    if _p not in sys.path:
        sys.path.insert(0, _p)

import concourse.mybir as mybir
from concourse import bacc
from concourse.tile import TileContext

B, T, C, H, D = 4, 2048, 2048, 16, 128
P = 128
NCORES = 8
HL = 8          # heads per core (local)
FL = HL * D     # local feature dim = 1024
NCC = C // P    # 16 contraction chunks
NTB = T // 512  # 4 query superblocks
NTC = T // P    # 16 t chunks
EXP_SCALE = 1.0 / math.sqrt(D)
NEG = -1.0e30

f32 = mybir.dt.float32
f16 = mybir.dt.float16


def build_nc():
    nc = bacc.Bacc()
    xt_d = nc.declare_dram_parameter("xt", [C, T], f16, isOutput=False)
    # wqk packed host-side as [jc, p, cc, col]: jc 0..7 = q head jc,
    # jc 8..15 = k head jc-8; one contiguous [128, 16*128] DMA per jc.
    wqk_d = nc.declare_dram_parameter("wqk", [16, P, NCC, P], f16, isOutput=False)
    wv_d = nc.declare_dram_parameter("wv", [NCC, P, FL], f16, isOutput=False)
    wp_d = nc.declare_dram_parameter("wp", [FL, C], f16, isOutput=False)
    masks_d = nc.declare_dram_parameter("masks", [P, P], f32, isOutput=False)
    ones_d = nc.declare_dram_parameter("ones", [P, P], f16, isOutput=False)
    out_d = nc.declare_dram_parameter("out", [T, C], f16, isOutput=True)

    ACT = mybir.ActivationFunctionType

    with TileContext(nc) as tc:
        with tc.tile_pool(name="const", bufs=1) as const_pool, \
             tc.tile_pool(name="qkt", bufs=1) as qkt_pool, \
             tc.tile_pool(name="vv", bufs=1) as v_pool:
            mask_sb = const_pool.tile([P, P], f32)
            nc.gpsimd.dma_start(mask_sb[:], masks_d[:])
            ones_sb = const_pool.tile([P, P], f16)
            nc.gpsimd.dma_start(ones_sb[:], ones_d[:])
            # dummy broadcast: loads the gpsimd Q7 library during phase 1 so
            # the first real phase-2 broadcast doesn't eat the ~6us load
            scratch_bc = const_pool.tile([P, 8], f32)
            nc.gpsimd.partition_broadcast(scratch_bc[:], mask_sb[0:1, 0:8])

            qkts = [qkt_pool.tile([P, T], f16, tag=f"qkt{j}", name=f"qkt{j}")
                    for j in range(16)]
            v_all = v_pool.tile([P, NTC, FL], f16, tag="vall")

            # ---------------- Phase 1: QKV projection ----------------
            with tc.tile_pool(name="xtp", bufs=1) as xt_pool:
                xts = [xt_pool.tile([P, T], f16, tag=f"xt{cc}", name=f"xt{cc}")
                       for cc in range(NCC)]

                with tc.tile_pool(name="ps1", bufs=8, space="PSUM") as ps1, \
                     tc.tile_pool(name="wv1", bufs=1) as wv_pool:
                    wvts = []
                    with tc.tile_pool(name="w1", bufs=3) as w_pool:
                        # q0/k0 weight chunks first so head 0's attention
                        # inputs exist as early as possible (jc 0 = q head 0,
                        # jc 8 = k head 0).
                        w01 = []
                        for jc in (0, 8):
                            wt_ = w_pool.tile([P, NCC, P], f16, tag="wqk",
                                              name=f"wqk{jc}")
                            w01.append(wt_)
                        # scalar queue interleave: tiny first chunks (cc0 of
                        # q0/k0 + first 512 xt cols) so the very first matmul
                        # can fire ~2.5us earlier, then the rest in rate order
                        # so both input streams flow from t=0 and the PE's
                        # chunk consumption is never starved
                        # everything the first ~15us of matmuls consume goes
                        # on the scalar queue in exact first-use order (the
                        # DMA engines drain it preferentially; a sync-queue
                        # chunk was observed arriving 11us late)
                        nc.scalar.dma_start(w01[0][:, 0:1, :], wqk_d[0, :, 0:1, :])
                        nc.scalar.dma_start(xts[0][:, 0:512], xt_d[0:P, 0:512])
                        nc.scalar.dma_start(w01[1][:, 0:1, :], wqk_d[8, :, 0:1, :])
                        nc.scalar.dma_start(xts[0][:, 512:1024],
                                            xt_d[0:P, 512:1024])
                        nc.scalar.dma_start(w01[0][:, 1:4, :], wqk_d[0, :, 1:4, :])
                        nc.scalar.dma_start(xts[0][:, 1024:1536],
                                            xt_d[0:P, 1024:1536])
                        nc.scalar.dma_start(w01[1][:, 1:4, :], wqk_d[8, :, 1:4, :])
                        nc.scalar.dma_start(xts[0][:, 1536:T],
                                            xt_d[0:P, 1536:T])
                        nc.scalar.dma_start(xts[1][:], xt_d[P:2 * P, :])
                        nc.sync.dma_start(xts[2][:], xt_d[2 * P:3 * P, :])
                        nc.scalar.dma_start(xts[3][:], xt_d[3 * P:4 * P, :])
                        nc.scalar.dma_start(w01[0][:, 4:NCC, :],
                                            wqk_d[0, :, 4:NCC, :])
                        nc.scalar.dma_start(w01[1][:, 4:NCC, :],
                                            wqk_d[8, :, 4:NCC, :])
                        for cc in range(4, NCC):
                            eng = nc.sync if cc % 2 == 0 else nc.scalar
                            eng.dma_start(
                                xts[cc][:], xt_d[cc * P:(cc + 1) * P, :])
                        # PE warm-up: the HAM clock gate only reaches 8/8
                        # after ~3.4us of sustained matmul activity, and the
                        # first real matmuls are DMA-starved until ~14us, so
                        # without this the whole first ~10us of real work runs
                        # at 1.2 GHz. Dummy matmuls on a memset tile (no DMA
                        # dependency) fill the DMA-wait dead time instead.
                        warm_sb = const_pool.tile([P, 512], f16,
                                                  name="warm_sb")
                        nc.vector.memset(warm_sb[:], 0)
                        warm_ps = ps1.tile([P, 512], f32, tag="ps1",
                                           name="warm_ps")
                        for wu in range(24):
                            nc.tensor.matmul(
                                warm_ps[:], warm_sb[:, 0:P], warm_sb[:],
                                start=(wu == 0), stop=(wu == 23))

                        # jc 0/8 emitted cc-outer: the PE's in-order queue
                        # then tracks xt chunk arrivals instead of blocking
                        # on the full xt load.
                        ps01 = [ps1.tile([P, 512], f32, tag="ps1",
                                         name=f"ps01_{g}") for g in range(8)]
                        for cc in range(NCC):
                            for wi in range(2):
                                for tb in range(NTB):
                                    nc.tensor.matmul(
                                        ps01[wi * NTB + tb][:],
                                        w01[wi][:, cc, :],
                                        xts[cc][:, tb * 512:(tb + 1) * 512],
                                        start=(cc == 0), stop=(cc == NCC - 1))
                        for wi, jc in enumerate((0, 8)):
                            for tb in range(NTB):
                                nc.scalar.copy(
                                    out=qkts[jc][:, tb * 512:(tb + 1) * 512],
                                    in_=ps01[wi * NTB + tb][:])

                        for jc in [j for j in range(16) if j not in (0, 8)]:
                            wt_ = w_pool.tile([P, NCC, P], f16, tag="wqk")
                            nc.scalar.dma_start(wt_[:], wqk_d[jc])
                            # cc-outer / tb-inner: 4 consecutive matmuls share
                            # the same stationary tile
                            pss = [ps1.tile([P, 512], f32, tag="ps1",
                                            name=f"psj{jc}_{tb}")
                                   for tb in range(NTB)]
                            for cc in range(NCC):
                                for tb in range(NTB):
                                    nc.tensor.matmul(
                                        pss[tb][:], wt_[:, cc, :],
                                        xts[cc][:, tb * 512:(tb + 1) * 512],
                                        start=(cc == 0), stop=(cc == NCC - 1))
                            for tb in range(NTB):
                                nc.scalar.copy(
                                    out=qkts[jc][:, tb * 512:(tb + 1) * 512],
                                    in_=pss[tb][:])

                        # wv loads stream behind the wqk stream during P1a so
                        # phase 1b starts without a handoff stall
                        for cc in range(NCC):
                            wt_ = wv_pool.tile([P, FL], f16, tag=f"wv{cc}",
                                               name=f"wv{cc}")
                            nc.scalar.dma_start(wt_[:], wv_d[cc])
                            wvts.append(wt_)

                    # 1b: v in natural [t, d] layout
                    if True:
                        for vb in range(FL // 512):
                            for tch in range(NTC):
                                ps = ps1.tile([P, 512], f32, tag="ps1")
                                for cc in range(NCC):
                                    nc.tensor.matmul(
                                        ps[:],
                                        xts[cc][:, tch * P:(tch + 1) * P],
                                        wvts[cc][:, vb * 512:(vb + 1) * 512],
                                        start=(cc == 0), stop=(cc == NCC - 1))
                                nc.vector.tensor_copy(
                                    out=v_all[:, tch, vb * 512:(vb + 1) * 512],
                                    in_=ps[:])

            # ---------------- Phase 2: attention ----------------
            with tc.tile_pool(name="avt", bufs=1) as avt_pool, \
                 tc.tile_pool(name="wpp", bufs=1) as wp_pool:
                avts = [avt_pool.tile([P, T], f16, tag=f"avt{h}", name=f"avt{h}")
                        for h in range(HL)]
                wps = [wp_pool.tile([P, C], f16, tag=f"wp{f}", name=f"wp{f}")
                       for f in range(HL)]

                with tc.tile_pool(name="p2sb", bufs=1) as p2sb, \
                     tc.tile_pool(name="ps2", bufs=1, space="PSUM") as ps2:

                    class Grp:
                        __slots__ = ("h", "si", "njc", "qt", "kt", "av_ps",
                                     "sumacc", "recb", "last_et", "last_infos",
                                     "first_et", "first_infos")

                    def si_group(h, si, qt, kt):
                        g = Grp()
                        g.h, g.si, g.njc, g.qt, g.kt = h, si, 4 * si + 4, qt, kt
                        g.av_ps = ps2.tile([P, 512], f32, tag="av",
                                           bufs=2, name=f"av_{h}_{si}")
                        # [key-part, chunk-parity x query]: clean pairs fold in
                        # with ONE full-width DVE add; the den ones-matmul sums
                        # both halves (PE has slack, DVE is the bottleneck).
                        # si 0 has a single non-last pair: its den reads both
                        # et tiles directly -- no sumacc, no DVE memset/adds.
                        g.sumacc = None if si == 0 else \
                            p2sb.tile([P, 1024], f16, tag="sum", bufs=3)
                        g.recb = None
                        return [(g, pp) for pp in range(g.njc // 2)]

                    def weave(a, b, frac=0.55):
                        """Spread the short group b inside the FIRST `frac` of
                        the long group a: the pipeline never runs shallow,
                        diagonal (masked, partial-width) pairs are kept apart,
                        and the two groups' close-out chains (den/recip/
                        broadcast) finish >=3 pairs apart so they never
                        serialize on gpsimd at a block boundary."""
                        out, bi = [], 0
                        na, nb = len(a), len(b)
                        for i, r in enumerate(a):
                            out.append(r)
                            while bi < nb and (bi + 1) * na * frac <= \
                                    (i + 1) * nb:
                                out.append(b[bi])
                                bi += 1
                        out.extend(b[bi:])
                        return out

                    # wp loads drip-fed on the (otherwise idle in phase 2)
                    # sync queue after the phase boundary; one per AV pair
                    # during head 1 so no single burst delays anything
                    wpq = [f for f in range(HL)]

                    def gen_pairs():
                        """Yield per-pair emission records across all
                        (head, si); scores run LOOK pairs ahead of AV."""
                        for h in range(HL):
                            qt, kt = qkts[h], qkts[8 + h]
                            # group CREATION order (3,1,0,2) sets the av_ps
                            # bank rotation: si0 inherits si3's bank (first
                            # reused 3 pairs into block 2) and si2 inherits
                            # si1's long-released bank, so no first-AV ever
                            # waits on a close-out multiply that just fired
                            # (the emission order itself is unchanged).
                            a = si_group(h, 3, qt, kt)
                            b = si_group(h, 1, qt, kt)
                            e = si_group(h, 0, qt, kt)
                            c = si_group(h, 2, qt, kt)
                            recs = weave(a, b)
                            recs += weave(c, e, frac=0.5)
                            yield from recs

                    def emit_sc(rec):
                        g, pp = rec
                        si = g.si
                        sc = ps2.tile([P, 1024], f32, tag="sc", bufs=3)
                        et = p2sb.tile([P, 1024], f16, tag="et", bufs=10)
                        infos = []
                        for jj in (0, 1):
                            j = 2 * pp + jj
                            diag = j * P - si * 512
                            d_off = max(0, diag)
                            # left-align the second diagonal block so the
                            # pair's valid region is contiguous -> one exp
                            base = jj * 512 - (d_off if jj else 0)
                            nc.tensor.matmul(
                                sc[:, base + d_off:base + 512],
                                g.kt[:, j * P:(j + 1) * P],
                                g.qt[:, si * 512 + d_off:(si + 1) * 512],
                                start=True, stop=True)
                            infos.append((j, diag >= 0, d_off, base))
                        # masks immediately after the score matmuls so exp is
                        # never queued behind lower-priority DVE work. A pair
                        # whose blocks are BOTH diagonal merges its two mask
                        # adds into ONE strided DVE op: with left-aligned
                        # bases the regions always sit at d_off0 and 512 with
                        # equal 128 widths (halves the mask-chain latency in
                        # front of the exp).
                        if infos[0][1] and infos[1][1]:
                            d0 = infos[0][2]
                            stw = 512 - d0
                            reg = sc[:, d0:d0 + 2 * stw].rearrange(
                                "p (b w) -> p b w", b=2)[:, :, 0:P]
                            mb = mask_sb[:].unsqueeze(1).broadcast_to(
                                (P, 2, P))
                            nc.vector.tensor_add(out=reg, in0=reg, in1=mb)
                        else:
                            for (j, is_diag, d_off, base) in infos:
                                if is_diag:
                                    nc.vector.tensor_add(
                                        out=sc[:, base + d_off:
                                               base + d_off + P],
                                        in0=sc[:, base + d_off:
                                               base + d_off + P],
                                        in1=mask_sb[:])
                        lo = infos[0][2] + infos[0][3]  # d_off0 (+ base0 == 0)
                        hi = infos[1][3] + 512          # base1 + 512
                        nc.scalar.activation(et[:, lo:hi], sc[:, lo:hi],
                                             ACT.Exp, scale=EXP_SCALE)
                        # flush the previous pair's lazily-queued sums now
                        # that this pair's masks+exp are already in the FIFOs
                        while sum_q:
                            sum_q.popleft()()
                        last_pair = (2 * pp + 1 == g.njc - 1)
                        if not last_pair:
                            # denominator accumulation trails the exps; it
                            # only gates the si-end ones-matmuls, never the
                            # next exp. Sums are emitted LAZILY (one pair
                            # later, via sum_q) so the next pair's exp-gating
                            # mask adds never queue behind them in the DVE
                            # FIFO. First (clean) pair initializes sumacc
                            # with a 4x-mode copy; later clean pairs fold in
                            # with one full-width 2x add; diagonal pairs add
                            # their two left-aligned valid regions into the
                            # matching sumacc halves.
                            clean = not (infos[0][1] or infos[1][1])
                            if si == 0:
                                g.first_et, g.first_infos = et, infos
                            elif pp == 0:
                                sum_q.append(lambda g=g, et=et:
                                             nc.vector.tensor_copy(
                                                 out=g.sumacc[:], in_=et[:]))
                            elif clean:
                                sum_q.append(lambda g=g, et=et:
                                             nc.vector.tensor_add(
                                                 out=g.sumacc[:],
                                                 in0=g.sumacc[:],
                                                 in1=et[:]))
                            else:
                                def diag_sums(g=g, et=et, infos=infos):
                                    for (j, is_diag, d_off, base) in infos:
                                        half = (j % 2) * 512
                                        nc.vector.tensor_add(
                                            out=g.sumacc[:, half + d_off:
                                                         half + 512],
                                            in0=g.sumacc[:, half + d_off:
                                                         half + 512],
                                            in1=et[:, base + d_off:
                                                   base + 512])
                                sum_q.append(diag_sums)
                        else:
                            g.last_et, g.last_infos = et, infos
                        return (rec, et, infos, last_pair)

                    def emit_den(g):
                        # Softmax close-out, emitted DEN_DELAY pairs after the
                        # group's last score pair -- i.e. *well before* the
                        # group's last AV matmuls (which trail by LOOK pairs).
                        # It depends only on exps/sumacc, so the reciprocal
                        # and its gpsimd broadcast overlap the AV stream and
                        # the final rescale multiply never stalls the PE.
                        # The last pair's exps are summed directly by PE
                        # ones-matmuls (PSUM accumulation) to spare DVE.
                        # 1-column ones stationary: a wider (broadcasting)
                        # ones matmul trips the P0 power downclock (PE drops
                        # 2.4->2.0 GHz for the whole kernel). The broadcast
                        # rides gpsimd, whose Q7 library was preloaded in
                        # phase 1 so no load stall lands here.
                        den = ps2.tile([P, 1024], f32, tag="sc", bufs=3,
                                       name=f"den_{g.h}_{g.si}")
                        if g.si == 0:
                            # no sumacc: read both pairs' ets directly. The
                            # first region (j0, d_off=0) is full width, so it
                            # anchors the accumulation group.
                            regions = [(g.first_et, inf) for inf in
                                       g.first_infos] + \
                                      [(g.last_et, inf) for inf in
                                       g.last_infos]
                            for ri, (et_, (j, is_diag, d_off, base)) in \
                                    enumerate(regions):
                                nc.tensor.matmul(
                                    den[0:1, d_off:512], ones_sb[:, 0:1],
                                    et_[:, base + d_off:base + 512],
                                    start=(ri == 0), stop=(j == g.njc - 1))
                        else:
                            nc.tensor.matmul(
                                den[0:1, 0:512], ones_sb[:, 0:1],
                                g.sumacc[:, 0:512], start=True, stop=False)
                            nc.tensor.matmul(
                                den[0:1, 0:512], ones_sb[:, 0:1],
                                g.sumacc[:, 512:1024], start=False, stop=False)
                            for (j, is_diag, d_off, base) in g.last_infos:
                                nc.tensor.matmul(
                                    den[0:1, d_off:512], ones_sb[:, 0:1],
                                    g.last_et[:, base + d_off:base + 512],
                                    start=False, stop=(j == g.njc - 1))
                        rec32 = p2sb.tile([1, 512], f32, tag="rec32",
                                          bufs=3)
                        nc.vector.reciprocal_approx_fast(
                            out=rec32[:], in_=den[0:1, 0:512])
                        recb = p2sb.tile([P, 512], f32, tag="recb",
                                         bufs=3)
                        nc.gpsimd.partition_broadcast(recb[:], rec32[:])
                        g.recb = recb

                    def emit_av(sc_rec):
                        (g, pp), et, infos, last_pair = sc_rec
                        h, si = g.h, g.si
                        if h >= 1 and wpq:
                            f = wpq.pop(0)
                            nc.sync.dma_start(
                                wps[f][:], wp_d[f * P:(f + 1) * P, :])
                        for (j, is_diag, d_off, base) in infos:
                            nc.tensor.matmul(
                                g.av_ps[:, d_off:],
                                v_all[:, j, h * P:(h + 1) * P],
                                et[:, base + d_off:base + 512],
                                start=(j == 0), stop=(j == g.njc - 1))
                        if last_pair:
                            if g.recb is None:
                                emit_den(g)
                            nc.vector.tensor_mul(
                                out=avts[h][:, si * 512:(si + 1) * 512],
                                in0=g.av_ps[:], in1=g.recb[:])

                    LOOK = 7
                    DEN_DELAY = 4  # effective delay: DEN_DELAY-1 sc pairs
                    pend = deque()
                    denq = deque()  # (group, countdown to den emission)
                    sum_q = deque()  # lazily-emitted sum thunks
                    for rec in gen_pairs():
                        sc_rec = emit_sc(rec)
                        pend.append(sc_rec)
                        if sc_rec[3]:
                            denq.append([sc_rec[0][0], DEN_DELAY])
                        for e in denq:
                            e[1] -= 1
                        while denq and denq[0][1] <= 0:
                            emit_den(denq.popleft()[0])
                        if len(pend) > LOOK:
                            emit_av(pend.popleft())
                    while sum_q:
                        sum_q.popleft()()
                    while denq:
                        emit_den(denq.popleft()[0])
                    while pend:
                        emit_av(pend.popleft())

                    # ------------ Phase 3: output projection ------------
                    # Merged into the phase-2 pool scope (PSUM tiles reuse
                    # tag "sc") so there is no pool-transition stall and no
                    # PE idle window for a HAM rethrottle. tch order follows
                    # avts readiness (head 7 closes si1, si3, si0, si2) so
                    # the first projection matmuls only depend on muls that
                    # fired pairs ago, and the late close-outs drain while
                    # the PE is already projecting.
                    p3_order = [4, 5, 6, 7, 12, 13, 14, 15,
                                0, 1, 2, 3, 8, 9, 10, 11]
                    for oi, tch in enumerate(p3_order):
                        ob = p2sb.tile([P, C], f16, tag="ocb", bufs=2)
                        for hb in range(2):
                            ps = ps2.tile([P, 1024], f32, tag="sc", bufs=3,
                                          name=f"p3_{tch}_{hb}")
                            for half in range(2):
                                cb_ = hb * 2 + half
                                for f in range(HL):
                                    nc.tensor.matmul(
                                        ps[:, half * 512:(half + 1) * 512],
                                        avts[f][:, tch * P:(tch + 1) * P],
                                        wps[f][:, cb_ * 512:(cb_ + 1) * 512],
                                        start=(f == 0), stop=(f == HL - 1))
                            # one wide copy per [P,1024] accumulation tile
                            if hb == 0:
                                nc.scalar.copy(
                                    out=ob[:, 0:1024], in_=ps[:])
                            else:
                                nc.vector.tensor_copy(
                                    out=ob[:, 1024:2048], in_=ps[:])
                        if oi >= NTC - 2:
                            # split the final transfers so the drain tail is
                            # one small DMA, not a 1MB one
                            for cb_ in range(C // 512):
                                eng = nc.sync if cb_ % 2 == 0 else nc.scalar
                                eng.dma_start(
                                    out_d[tch * P:(tch + 1) * P,
                                          cb_ * 512:(cb_ + 1) * 512],
                                    ob[:, cb_ * 512:(cb_ + 1) * 512])
                        else:
                            eng = nc.sync if oi % 2 == 0 else nc.scalar
                            eng.dma_start(
                                out_d[tch * P:(tch + 1) * P, :], ob[:])
    nc.compile()
    return nc


def _make_masks():
    pp = np.arange(P)[:, None]
    ff = np.arange(P)[None, :]
    return np.where(ff >= pp, 0.0, NEG).astype(np.float32)


def _prep_inputs(x, w_qkv, w_proj):
    masks = _make_masks()
    per_g = {}
    for g in range(2):
        cols = slice(g * FL, (g + 1) * FL)
        wqk_c = np.concatenate(
            [w_qkv[:, :C][:, cols], w_qkv[:, C:2 * C][:, cols]], axis=1)
        # [jc, p, cc, col]: one contiguous DMA per jc
        wqk_packed = np.ascontiguousarray(
            wqk_c.reshape(NCC, P, 16, P).transpose(2, 1, 0, 3)).astype(np.float16)
        wv_c = np.ascontiguousarray(
            w_qkv[:, 2 * C:][:, cols].reshape(NCC, P, FL)).astype(np.float16)
        wp_c = np.ascontiguousarray(
            w_proj[g * FL:(g + 1) * FL, :]).astype(np.float16)
        per_g[g] = (wqk_packed, wv_c, wp_c)
    in_maps = []
    for core in range(NCORES):
        b, g = core // 2, core % 2
        wqk_packed, wv_c, wp_c = per_g[g]
        in_maps.append({
            "xt": np.ascontiguousarray(x[b].T).astype(np.float16),
            "wqk": wqk_packed,
            "wv": wv_c,
            "wp": wp_c,
            "masks": masks,
            "ones": np.ones((P, P), dtype=np.float16),
        })
    return in_maps


_nc_cache = None
last_results = None  # BassKernelResults of the most recent run (for test.py)


def kernel(x, w_qkv, w_proj):
    global _nc_cache, last_results
    from concourse.bass_utils import run_bass_kernel_spmd

    x = np.asarray(x, dtype=np.float32)
    w_qkv = np.asarray(w_qkv, dtype=np.float32)
    w_proj = np.asarray(w_proj, dtype=np.float32)

    if _nc_cache is None:
        _nc_cache = build_nc()
    nc = _nc_cache

    in_maps = _prep_inputs(x, w_qkv, w_proj)
    trace = bool(int(os.environ.get("KERNEL_TRACE", "0")))
    res = run_bass_kernel_spmd(nc, in_maps, list(range(NCORES)), trace=trace)
    last_results = res

    out = np.empty((B, T, C), dtype=np.float32)
    for b in range(B):
        out[b] = (res.results[2 * b]["out"].astype(np.float32)
                  + res.results[2 * b + 1]["out"].astype(np.float32))
    return out



# revision 53
# speedup vs baseline: 1.1928x; 1.1928x over previous
"""Causal self-attention Trainium2 Bass kernel (fp16 SBUF-resident rewrite).

Problem: B=4, T=2048, C=2048, H=16 heads, D=128 head dim, fp32 in/out.
  qkv = x @ w_qkv ; causal softmax(q k^T / sqrt(D)) v ; out = av @ w_proj

Sharding (8 NeuronCores): DP=4 over batch x TP=2 over head groups
(Megatron-style: w_qkv columns / w_proj rows split by heads). Core
c handles batch b=c//2, heads g*8..g*8+8 with g=c%2. Each core emits a
partial [T, C] f32 output; host sums the TP pairs.

Key design points (in rough order of discovery):
  * All operands stored fp16 (inputs cast host-side); matmuls run fp16
    at the same 1 cycle/row PE rate as f32r but with half the SBUF/DMA
    traffic, so q/k/v/av stay SBUF-resident end to end (no DRAM round
    trip between phases).
  * Exp runs in [128,1024] two-PSUM-bank mega tiles (halves the ACT
    per-instruction access overhead); a diagonal pair left-aligns its
    second block so the pair is one contiguous exp with no garbage.
  * Phase 2 is one globally software-pipelined stream of block-pairs:
    scores run LOOK=7 pairs ahead of AV, and the softmax close-out
    chain (den ones-matmuls -> DVE reciprocal -> gpsimd broadcast) is
    emitted DEN_DELAY-1 pairs after a group's last score pair, i.e.
    several pairs BEFORE its last AV matmuls, so the final rescale
    multiply never stalls the PE or convoys the DVE FIFO.
  * Sum accumulation is [128,1024]-wide: clean pairs fold in with ONE
    2x-mode DVE add, pair 0 initializes via a 4x-mode copy, and sums
    are emitted lazily (one pair late) so the exp-gating mask adds
    never queue behind them. si0 skips sumacc entirely (its den reads
    both et tiles directly). The den ones-matmul must use a 1-column
    stationary: a 128-column (pre-broadcast) ones matmul trips the P0
    power downclock (PE 2.4 -> 2.0 GHz for the entire kernel).
  * gpsimd runs EXACTLY ONE compute op type (partition_broadcast):
    mixing op types reloads the Q7 library at ~6us per switch. A dummy
    broadcast in phase 1 preloads the library off the critical path.
  * Blocks are woven (si3,si1)+(si2,si0) with the short group placed in
    the first ~55% so consecutive close-out chains never collide.
  * ~24 dummy matmuls on a memset tile warm the HAM clock gate (4/8 ->
    8/8 at 2.4 GHz) during the DMA-ramp dead time at kernel start, and
    the startup-critical x/w chunks ride one queue in first-use order.
  * Phase 3 is merged into the phase-2 pool scope (PSUM reuses tag
    "sc") with tch emitted in avts-readiness order, so projection
    matmuls start while the last head's close-outs drain and the PE
    never idles long enough for a HAM rethrottle at the seam.
  * Phase-3 partials are written f16 (summed in f32 on host); wp loads
    are drip-fed on the idle sync queue during phase 2.
"""

import math
import os
import sys
from collections import deque

import numpy as np

for _p in ("/opt/trn_rl_repo",):
    if _p not in sys.path:
        sys.path.insert(0, _p)

import concourse.mybir as mybir
from concourse import bacc
from concourse.tile import TileContext

B, T, C, H, D = 4, 2048, 2048, 16, 128
P = 128
NCORES = 8
HL = 8          # heads per core (local)
FL = HL * D     # local feature dim = 1024
NCC = C // P    # 16 contraction chunks
NTB = T // 512  # 4 query superblocks
NTC = T // P    # 16 t chunks
EXP_SCALE = 1.0 / math.sqrt(D)
NEG = -1.0e30

f32 = mybir.dt.float32
f16 = mybir.dt.float16


def build_nc():
    nc = bacc.Bacc()
    xt_d = nc.declare_dram_parameter("xt", [C, T], f16, isOutput=False)
    # wqk packed host-side as [jc, p, cc, col]: jc 0..7 = q head jc,
    # jc 8..15 = k head jc-8; one contiguous [128, 16*128] DMA per jc.
    wqk_d = nc.declare_dram_parameter("wqk", [16, P, NCC, P], f16, isOutput=False)
    wv_d = nc.declare_dram_parameter("wv", [NCC, P, FL], f16, isOutput=False)
    wp_d = nc.declare_dram_parameter("wp", [FL, C], f16, isOutput=False)
    masks_d = nc.declare_dram_parameter("masks", [P, P], f32, isOutput=False)
    ones_d = nc.declare_dram_parameter("ones", [P, P], f16, isOutput=False)
    out_d = nc.declare_dram_parameter("out", [T, C], f16, isOutput=True)

    ACT = mybir.ActivationFunctionType

    with TileContext(nc) as tc:
        with tc.tile_pool(name="const", bufs=1) as const_pool, \
             tc.tile_pool(name="qkt", bufs=1) as qkt_pool, \
             tc.tile_pool(name="vv", bufs=1) as v_pool:
            mask_sb = const_pool.tile([P, P], f32)
            nc.gpsimd.dma_start(mask_sb[:], masks_d[:])
            ones_sb = const_pool.tile([P, P], f16)
            nc.gpsimd.dma_start(ones_sb[:], ones_d[:])
            # dummy broadcast: loads the gpsimd Q7 library during phase 1 so
            # the first real phase-2 broadcast doesn't eat the ~6us load
            scratch_bc = const_pool.tile([P, 8], f32)
            nc.gpsimd.partition_broadcast(scratch_bc[:], mask_sb[0:1, 0:8])

            qkts = [qkt_pool.tile([P, T], f16, tag=f"qkt{j}", name=f"qkt{j}")
                    for j in range(16)]
            v_all = v_pool.tile([P, NTC, FL], f16, tag="vall")

            # ---------------- Phase 1: QKV projection ----------------
            with tc.tile_pool(name="xtp", bufs=1) as xt_pool:
                xts = [xt_pool.tile([P, T], f16, tag=f"xt{cc}", name=f"xt{cc}")
                       for cc in range(NCC)]

                with tc.tile_pool(name="ps1", bufs=8, space="PSUM") as ps1, \
                     tc.tile_pool(name="wv1", bufs=1) as wv_pool:
                    wvts = []
                    with tc.tile_pool(name="w1", bufs=3) as w_pool:
                        # q0/k0 weight chunks first so head 0's attention
                        # inputs exist as early as possible (jc 0 = q head 0,
                        # jc 8 = k head 0).
                        w01 = []
                        for jc in (0, 8):
                            wt_ = w_pool.tile([P, NCC, P], f16, tag="wqk",
                                              name=f"wqk{jc}")
                            w01.append(wt_)
                        # scalar queue interleave: tiny first chunks (cc0 of
                        # q0/k0 + first 512 xt cols) so the very first matmul
                        # can fire ~2.5us earlier, then the rest in rate order
                        # so both input streams flow from t=0 and the PE's
                        # chunk consumption is never starved
                        # everything the first ~15us of matmuls consume goes
                        # on the scalar queue in exact first-use order (the
                        # DMA engines drain it preferentially; a sync-queue
                        # chunk was observed arriving 11us late)
                        nc.scalar.dma_start(w01[0][:, 0:1, :], wqk_d[0, :, 0:1, :])
                        nc.scalar.dma_start(xts[0][:, 0:512], xt_d[0:P, 0:512])
                        nc.scalar.dma_start(w01[1][:, 0:1, :], wqk_d[8, :, 0:1, :])
                        nc.scalar.dma_start(xts[0][:, 512:1024],
                                            xt_d[0:P, 512:1024])
                        nc.scalar.dma_start(w01[0][:, 1:4, :], wqk_d[0, :, 1:4, :])
                        nc.scalar.dma_start(xts[0][:, 1024:1536],
                                            xt_d[0:P, 1024:1536])
                        nc.scalar.dma_start(w01[1][:, 1:4, :], wqk_d[8, :, 1:4, :])
                        nc.scalar.dma_start(xts[0][:, 1536:T],
                                            xt_d[0:P, 1536:T])
                        nc.scalar.dma_start(xts[1][:], xt_d[P:2 * P, :])
                        nc.sync.dma_start(xts[2][:], xt_d[2 * P:3 * P, :])
                        nc.scalar.dma_start(xts[3][:], xt_d[3 * P:4 * P, :])
                        nc.scalar.dma_start(w01[0][:, 4:NCC, :],
                                            wqk_d[0, :, 4:NCC, :])
                        nc.scalar.dma_start(w01[1][:, 4:NCC, :],
                                            wqk_d[8, :, 4:NCC, :])
                        for cc in range(4, NCC):
                            eng = nc.sync if cc % 2 == 0 else nc.scalar
                            eng.dma_start(
                                xts[cc][:], xt_d[cc * P:(cc + 1) * P, :])
                        # PE warm-up: the HAM clock gate only reaches 8/8
                        # after ~3.4us of sustained matmul activity, and the
                        # first real matmuls are DMA-starved until ~14us, so
                        # without this the whole first ~10us of real work runs
                        # at 1.2 GHz. Dummy matmuls on a memset tile (no DMA
                        # dependency) fill the DMA-wait dead time instead.
                        warm_sb = const_pool.tile([P, 512], f16,
                                                  name="warm_sb")
                        nc.vector.memset(warm_sb[:], 0)
                        warm_ps = ps1.tile([P, 512], f32, tag="ps1",
                                           name="warm_ps")
                        for wu in range(24):
                            nc.tensor.matmul(
                                warm_ps[:], warm_sb[:, 0:P], warm_sb[:],
                                start=(wu == 0), stop=(wu == 23))

                        # jc 0/8 emitted cc-outer: the PE's in-order queue
                        # then tracks xt chunk arrivals instead of blocking
                        # on the full xt load.
                        ps01 = [ps1.tile([P, 512], f32, tag="ps1",
                                         name=f"ps01_{g}") for g in range(8)]
                        for cc in range(NCC):
                            for wi in range(2):
                                for tb in range(NTB):
                                    nc.tensor.matmul(
                                        ps01[wi * NTB + tb][:],
                                        w01[wi][:, cc, :],
                                        xts[cc][:, tb * 512:(tb + 1) * 512],
                                        start=(cc == 0), stop=(cc == NCC - 1))
                        for wi, jc in enumerate((0, 8)):
                            for tb in range(NTB):
                                nc.scalar.copy(
                                    out=qkts[jc][:, tb * 512:(tb + 1) * 512],
                                    in_=ps01[wi * NTB + tb][:])

                        for jc in [j for j in range(16) if j not in (0, 8)]:
                            wt_ = w_pool.tile([P, NCC, P], f16, tag="wqk")
                            nc.scalar.dma_start(wt_[:], wqk_d[jc])
                            # cc-outer / tb-inner: 4 consecutive matmuls share
                            # the same stationary tile
                            pss = [ps1.tile([P, 512], f32, tag="ps1",
                                            name=f"psj{jc}_{tb}")
                                   for tb in range(NTB)]
                            for cc in range(NCC):
                                for tb in range(NTB):
                                    nc.tensor.matmul(
                                        pss[tb][:], wt_[:, cc, :],
                                        xts[cc][:, tb * 512:(tb + 1) * 512],
                                        start=(cc == 0), stop=(cc == NCC - 1))
                            for tb in range(NTB):
                                nc.scalar.copy(
                                    out=qkts[jc][:, tb * 512:(tb + 1) * 512],
                                    in_=pss[tb][:])

                        # wv loads stream behind the wqk stream during P1a so
                        # phase 1b starts without a handoff stall
                        for cc in range(NCC):
                            wt_ = wv_pool.tile([P, FL], f16, tag=f"wv{cc}",
                                               name=f"wv{cc}")
                            nc.scalar.dma_start(wt_[:], wv_d[cc])
                            wvts.append(wt_)

                    # 1b: v in natural [t, d] layout
                    if True:
                        for vb in range(FL // 512):
                            for tch in range(NTC):
                                ps = ps1.tile([P, 512], f32, tag="ps1")
                                for cc in range(NCC):
                                    nc.tensor.matmul(
                                        ps[:],
                                        xts[cc][:, tch * P:(tch + 1) * P],
                                        wvts[cc][:, vb * 512:(vb + 1) * 512],
                                        start=(cc == 0), stop=(cc == NCC - 1))
                                nc.vector.tensor_copy(
                                    out=v_all[:, tch, vb * 512:(vb + 1) * 512],
                                    in_=ps[:])

            # ---------------- Phase 2: attention ----------------
            with tc.tile_pool(name="avt", bufs=1) as avt_pool, \
                 tc.tile_pool(name="wpp", bufs=1) as wp_pool:
                avts = [avt_pool.tile([P, T], f16, tag=f"avt{h}", name=f"avt{h}")
                        for h in range(HL)]
                wps = [wp_pool.tile([P, C], f16, tag=f"wp{f}", name=f"wp{f}")
                       for f in range(HL)]

                with tc.tile_pool(name="p2sb", bufs=1) as p2sb, \
                     tc.tile_pool(name="ps2", bufs=1, space="PSUM") as ps2:

                    class Grp:
                        __slots__ = ("h", "si", "njc", "qt", "kt", "av_ps",
                                     "sumacc", "recb", "last_et", "last_infos",
                                     "first_et", "first_infos")

                    def si_group(h, si, qt, kt):
                        g = Grp()
                        g.h, g.si, g.njc, g.qt, g.kt = h, si, 4 * si + 4, qt, kt
                        g.av_ps = ps2.tile([P, 512], f32, tag="av",
                                           bufs=2, name=f"av_{h}_{si}")
                        # [key-part, chunk-parity x query]: clean pairs fold in
                        # with ONE full-width DVE add; the den ones-matmul sums
                        # both halves (PE has slack, DVE is the bottleneck).
                        # si 0 has a single non-last pair: its den reads both
                        # et tiles directly -- no sumacc, no DVE memset/adds.
                        g.sumacc = None if si == 0 else \
                            p2sb.tile([P, 1024], f16, tag="sum", bufs=3)
                        g.recb = None
                        return [(g, pp) for pp in range(g.njc // 2)]

                    def weave(a, b, frac=0.55):
                        """Spread the short group b inside the FIRST `frac` of
                        the long group a: the pipeline never runs shallow,
                        diagonal (masked, partial-width) pairs are kept apart,
                        and the two groups' close-out chains (den/recip/
                        broadcast) finish >=3 pairs apart so they never
                        serialize on gpsimd at a block boundary."""
                        out, bi = [], 0
                        na, nb = len(a), len(b)
                        for i, r in enumerate(a):
                            out.append(r)
                            while bi < nb and (bi + 1) * na * frac <= \
                                    (i + 1) * nb:
                                out.append(b[bi])
                                bi += 1
                        out.extend(b[bi:])
                        return out

                    # wp loads drip-fed on the (otherwise idle in phase 2)
                    # sync queue after the phase boundary; one per AV pair
                    # during head 1 so no single burst delays anything
                    wpq = [f for f in range(HL)]

                    def gen_pairs():
                        """Yield per-pair emission records across all
                        (head, si); scores run LOOK pairs ahead of AV."""
                        for h in range(HL):
                            qt, kt = qkts[h], qkts[8 + h]
                            recs = weave(si_group(h, 3, qt, kt),
                                         si_group(h, 1, qt, kt))
                            recs += weave(si_group(h, 2, qt, kt),
                                          si_group(h, 0, qt, kt), frac=0.5)
                            yield from recs

                    def emit_sc(rec):
                        g, pp = rec
                        si = g.si
                        sc = ps2.tile([P, 1024], f32, tag="sc", bufs=3)
                        et = p2sb.tile([P, 1024], f16, tag="et", bufs=10)
                        infos = []
                        for jj in (0, 1):
                            j = 2 * pp + jj
                            diag = j * P - si * 512
                            d_off = max(0, diag)
                            # left-align the second diagonal block so the
                            # pair's valid region is contiguous -> one exp
                            base = jj * 512 - (d_off if jj else 0)
                            nc.tensor.matmul(
                                sc[:, base + d_off:base + 512],
                                g.kt[:, j * P:(j + 1) * P],
                                g.qt[:, si * 512 + d_off:(si + 1) * 512],
                                start=True, stop=True)
                            infos.append((j, diag >= 0, d_off, base))
                        # masks immediately after the score matmuls so exp is
                        # never queued behind lower-priority DVE work. A pair
                        # whose blocks are BOTH diagonal merges its two mask
                        # adds into ONE strided DVE op: with left-aligned
                        # bases the regions always sit at d_off0 and 512 with
                        # equal 128 widths (halves the mask-chain latency in
                        # front of the exp).
                        if infos[0][1] and infos[1][1]:
                            d0 = infos[0][2]
                            stw = 512 - d0
                            reg = sc[:, d0:d0 + 2 * stw].rearrange(
                                "p (b w) -> p b w", b=2)[:, :, 0:P]
                            mb = mask_sb[:].unsqueeze(1).broadcast_to(
                                (P, 2, P))
                            nc.vector.tensor_add(out=reg, in0=reg, in1=mb)
                        else:
                            for (j, is_diag, d_off, base) in infos:
                                if is_diag:
                                    nc.vector.tensor_add(
                                        out=sc[:, base + d_off:
                                               base + d_off + P],
                                        in0=sc[:, base + d_off:
                                               base + d_off + P],
                                        in1=mask_sb[:])
                        lo = infos[0][2] + infos[0][3]  # d_off0 (+ base0 == 0)
                        hi = infos[1][3] + 512          # base1 + 512
                        nc.scalar.activation(et[:, lo:hi], sc[:, lo:hi],
                                             ACT.Exp, scale=EXP_SCALE)
                        # flush the previous pair's lazily-queued sums now
                        # that this pair's masks+exp are already in the FIFOs
                        while sum_q:
                            sum_q.popleft()()
                        last_pair = (2 * pp + 1 == g.njc - 1)
                        if not last_pair:
                            # denominator accumulation trails the exps; it
                            # only gates the si-end ones-matmuls, never the
                            # next exp. Sums are emitted LAZILY (one pair
                            # later, via sum_q) so the next pair's exp-gating
                            # mask adds never queue behind them in the DVE
                            # FIFO. First (clean) pair initializes sumacc
                            # with a 4x-mode copy; later clean pairs fold in
                            # with one full-width 2x add; diagonal pairs add
                            # their two left-aligned valid regions into the
                            # matching sumacc halves.
                            clean = not (infos[0][1] or infos[1][1])
                            if si == 0:
                                g.first_et, g.first_infos = et, infos
                            elif pp == 0:
                                sum_q.append(lambda g=g, et=et:
                                             nc.vector.tensor_copy(
                                                 out=g.sumacc[:], in_=et[:]))
                            elif clean:
                                sum_q.append(lambda g=g, et=et:
                                             nc.vector.tensor_add(
                                                 out=g.sumacc[:],
                                                 in0=g.sumacc[:],
                                                 in1=et[:]))
                            else:
                                def diag_sums(g=g, et=et, infos=infos):
                                    for (j, is_diag, d_off, base) in infos:
                                        half = (j % 2) * 512
                                        nc.vector.tensor_add(
                                            out=g.sumacc[:, half + d_off:
                                                         half + 512],
                                            in0=g.sumacc[:, half + d_off:
                                                         half + 512],
                                            in1=et[:, base + d_off:
                                                   base + 512])
                                sum_q.append(diag_sums)
                        else:
                            g.last_et, g.last_infos = et, infos
                        return (rec, et, infos, last_pair)

                    def emit_den(g):
                        # Softmax close-out, emitted DEN_DELAY pairs after the
                        # group's last score pair -- i.e. *well before* the
                        # group's last AV matmuls (which trail by LOOK pairs).
                        # It depends only on exps/sumacc, so the reciprocal
                        # and its gpsimd broadcast overlap the AV stream and
                        # the final rescale multiply never stalls the PE.
                        # The last pair's exps are summed directly by PE
                        # ones-matmuls (PSUM accumulation) to spare DVE.
                        # 1-column ones stationary: a wider (broadcasting)
                        # ones matmul trips the P0 power downclock (PE drops
                        # 2.4->2.0 GHz for the whole kernel). The broadcast
                        # rides gpsimd, whose Q7 library was preloaded in
                        # phase 1 so no load stall lands here.
                        den = ps2.tile([P, 1024], f32, tag="sc", bufs=3,
                                       name=f"den_{g.h}_{g.si}")
                        if g.si == 0:
                            # no sumacc: read both pairs' ets directly. The
                            # first region (j0, d_off=0) is full width, so it
                            # anchors the accumulation group.
                            regions = [(g.first_et, inf) for inf in
                                       g.first_infos] + \
                                      [(g.last_et, inf) for inf in
                                       g.last_infos]
                            for ri, (et_, (j, is_diag, d_off, base)) in \
                                    enumerate(regions):
                                nc.tensor.matmul(
                                    den[0:1, d_off:512], ones_sb[:, 0:1],
                                    et_[:, base + d_off:base + 512],
                                    start=(ri == 0), stop=(j == g.njc - 1))
                        else:
                            nc.tensor.matmul(
                                den[0:1, 0:512], ones_sb[:, 0:1],
                                g.sumacc[:, 0:512], start=True, stop=False)
                            nc.tensor.matmul(
                                den[0:1, 0:512], ones_sb[:, 0:1],
                                g.sumacc[:, 512:1024], start=False, stop=False)
                            for (j, is_diag, d_off, base) in g.last_infos:
                                nc.tensor.matmul(
                                    den[0:1, d_off:512], ones_sb[:, 0:1],
                                    g.last_et[:, base + d_off:base + 512],
                                    start=False, stop=(j == g.njc - 1))
                        rec32 = p2sb.tile([1, 512], f32, tag="rec32",
                                          bufs=3)
                        nc.vector.reciprocal_approx_fast(
                            out=rec32[:], in_=den[0:1, 0:512])
                        recb = p2sb.tile([P, 512], f32, tag="recb",
                                         bufs=3)
                        nc.gpsimd.partition_broadcast(recb[:], rec32[:])
                        g.recb = recb

                    def emit_av(sc_rec):
                        (g, pp), et, infos, last_pair = sc_rec
                        h, si = g.h, g.si
                        if h >= 1 and wpq:
                            f = wpq.pop(0)
                            nc.sync.dma_start(
                                wps[f][:], wp_d[f * P:(f + 1) * P, :])
                        for (j, is_diag, d_off, base) in infos:
                            nc.tensor.matmul(
                                g.av_ps[:, d_off:],
                                v_all[:, j, h * P:(h + 1) * P],
                                et[:, base + d_off:base + 512],
                                start=(j == 0), stop=(j == g.njc - 1))
                        if last_pair:
                            if g.recb is None:
                                emit_den(g)
                            nc.vector.tensor_mul(
                                out=avts[h][:, si * 512:(si + 1) * 512],
                                in0=g.av_ps[:], in1=g.recb[:])

                    LOOK = 7
                    DEN_DELAY = 4  # effective delay: DEN_DELAY-1 sc pairs
                    pend = deque()
                    denq = deque()  # (group, countdown to den emission)
                    sum_q = deque()  # lazily-emitted sum thunks
                    for rec in gen_pairs():
                        sc_rec = emit_sc(rec)
                        pend.append(sc_rec)
                        if sc_rec[3]:
                            denq.append([sc_rec[0][0], DEN_DELAY])
                        for e in denq:
                            e[1] -= 1
                        while denq and denq[0][1] <= 0:
                            emit_den(denq.popleft()[0])
                        if len(pend) > LOOK:
                            emit_av(pend.popleft())
                    while sum_q:
                        sum_q.popleft()()
                    while denq:
                        emit_den(denq.popleft()[0])
                    while pend:
                        emit_av(pend.popleft())

                    # ------------ Phase 3: output projection ------------
                    # Merged into the phase-2 pool scope (PSUM tiles reuse
                    # tag "sc") so there is no pool-transition stall and no
                    # PE idle window for a HAM rethrottle. tch order follows
                    # avts readiness (head 7 closes si1, si3, si0, si2) so
                    # the first projection matmuls only depend on muls that
                    # fired pairs ago, and the late close-outs drain while
                    # the PE is already projecting.
                    p3_order = [4, 5, 6, 7, 12, 13, 14, 15,
                                0, 1, 2, 3, 8, 9, 10, 11]
                    for oi, tch in enumerate(p3_order):
                        ob = p2sb.tile([P, C], f16, tag="ocb", bufs=2)
                        for hb in range(2):
                            ps = ps2.tile([P, 1024], f32, tag="sc", bufs=3,
                                          name=f"p3_{tch}_{hb}")
                            for half in range(2):
                                cb_ = hb * 2 + half
                                for f in range(HL):
                                    nc.tensor.matmul(
                                        ps[:, half * 512:(half + 1) * 512],
                                        avts[f][:, tch * P:(tch + 1) * P],
                                        wps[f][:, cb_ * 512:(cb_ + 1) * 512],
                                        start=(f == 0), stop=(f == HL - 1))
                            # one wide copy per [P,1024] accumulation tile
                            if hb == 0:
                                nc.scalar.copy(
                                    out=ob[:, 0:1024], in_=ps[:])
                            else:
                                nc.vector.tensor_copy(
                                    out=ob[:, 1024:2048], in_=ps[:])
                        if oi >= NTC - 2:
                            # split the final transfers so the drain tail is
                            # one small DMA, not a 1MB one
                            for cb_ in range(C // 512):
                                eng = nc.sync if cb_ % 2 == 0 else nc.scalar
                                eng.dma_start(
                                    out_d[tch * P:(tch + 1) * P,
                                          cb_ * 512:(cb_ + 1) * 512],
                                    ob[:, cb_ * 512:(cb_ + 1) * 512])
                        else:
                            eng = nc.sync if oi % 2 == 0 else nc.scalar
                            eng.dma_start(
                                out_d[tch * P:(tch + 1) * P, :], ob[:])
    nc.compile()
    return nc


def _make_masks():
    pp = np.arange(P)[:, None]
    ff = np.arange(P)[None, :]
    return np.where(ff >= pp, 0.0, NEG).astype(np.float32)


def _prep_inputs(x, w_qkv, w_proj):
    masks = _make_masks()
    per_g = {}
    for g in range(2):
        cols = slice(g * FL, (g + 1) * FL)
        wqk_c = np.concatenate(
            [w_qkv[:, :C][:, cols], w_qkv[:, C:2 * C][:, cols]], axis=1)
        # [jc, p, cc, col]: one contiguous DMA per jc
        wqk_packed = np.ascontiguousarray(
            wqk_c.reshape(NCC, P, 16, P).transpose(2, 1, 0, 3)).astype(np.float16)
        wv_c = np.ascontiguousarray(
            w_qkv[:, 2 * C:][:, cols].reshape(NCC, P, FL)).astype(np.float16)
        wp_c = np.ascontiguousarray(
            w_proj[g * FL:(g + 1) * FL, :]).astype(np.float16)
        per_g[g] = (wqk_packed, wv_c, wp_c)
    in_maps = []
    for core in range(NCORES):
        b, g = core // 2, core % 2
        wqk_packed, wv_c, wp_c = per_g[g]
        in_maps.append({
            "xt": np.ascontiguousarray(x[b].T).astype(np.float16),
            "wqk": wqk_packed,
            "wv": wv_c,
            "wp": wp_c,
            "masks": masks,
            "ones": np.ones((P, P), dtype=np.float16),
        })
    return in_maps


_nc_cache = None
last_results = None  # BassKernelResults of the most recent run (for test.py)


def kernel(x, w_qkv, w_proj):
    global _nc_cache, last_results
    from concourse.bass_utils import run_bass_kernel_spmd

    x = np.asarray(x, dtype=np.float32)
    w_qkv = np.asarray(w_qkv, dtype=np.float32)
    w_proj = np.asarray(w_proj, dtype=np.float32)

    if _nc_cache is None:
        _nc_cache = build_nc()
    nc = _nc_cache

    in_maps = _prep_inputs(x, w_qkv, w_proj)
    trace = bool(int(os.environ.get("KERNEL_TRACE", "0")))
    res = run_bass_kernel_spmd(nc, in_maps, list(range(NCORES)), trace=trace)
    last_results = res

    out = np.empty((B, T, C), dtype=np.float32)
    for b in range(B):
        out[b] = (res.results[2 * b]["out"].astype(np.float32)
                  + res.results[2 * b + 1]["out"].astype(np.float32))
    return out



# revision 54
# speedup vs baseline: 1.1954x; 1.0021x over previous
"""Causal self-attention Trainium2 Bass kernel (fp16 SBUF-resident rewrite).

Problem: B=4, T=2048, C=2048, H=16 heads, D=128 head dim, fp32 in/out.
  qkv = x @ w_qkv ; causal softmax(q k^T / sqrt(D)) v ; out = av @ w_proj

Sharding (8 NeuronCores): DP=4 over batch x TP=2 over head groups
(Megatron-style: w_qkv columns / w_proj rows split by heads). Core
c handles batch b=c//2, heads g*8..g*8+8 with g=c%2. Each core emits a
partial [T, C] f32 output; host sums the TP pairs.

Key design points (in rough order of discovery):
  * All operands stored fp16 (inputs cast host-side); matmuls run fp16
    at the same 1 cycle/row PE rate as f32r but with half the SBUF/DMA
    traffic, so q/k/v/av stay SBUF-resident end to end (no DRAM round
    trip between phases).
  * Exp runs in [128,1024] two-PSUM-bank mega tiles (halves the ACT
    per-instruction access overhead); a diagonal pair left-aligns its
    second block so the pair is one contiguous exp with no garbage.
  * Phase 2 is one globally software-pipelined stream of block-pairs:
    scores run LOOK=7 pairs ahead of AV, and the softmax close-out
    chain (den ones-matmuls -> DVE reciprocal -> gpsimd broadcast) is
    emitted DEN_DELAY-1 pairs after a group's last score pair, i.e.
    several pairs BEFORE its last AV matmuls, so the final rescale
    multiply never stalls the PE or convoys the DVE FIFO.
  * Sum accumulation is [128,1024]-wide: clean pairs fold in with ONE
    2x-mode DVE add, pair 0 initializes via a 4x-mode copy, and sums
    are emitted lazily (one pair late) so the exp-gating mask adds
    never queue behind them. si0 skips sumacc entirely (its den reads
    both et tiles directly). The den ones-matmul must use a 1-column
    stationary: a 128-column (pre-broadcast) ones matmul trips the P0
    power downclock (PE 2.4 -> 2.0 GHz for the entire kernel).
  * gpsimd runs EXACTLY ONE compute op type (partition_broadcast):
    mixing op types reloads the Q7 library at ~6us per switch. A dummy
    broadcast in phase 1 preloads the library off the critical path.
  * Blocks are woven (si3,si1)+(si2,si0) with the short group placed in
    the first ~55% so consecutive close-out chains never collide.
  * ~24 dummy matmuls on a memset tile warm the HAM clock gate (4/8 ->
    8/8 at 2.4 GHz) during the DMA-ramp dead time at kernel start, and
    the startup-critical x/w chunks ride one queue in first-use order.
  * Phase 3 is merged into the phase-2 pool scope (PSUM reuses tag
    "sc") with tch emitted in avts-readiness order, so projection
    matmuls start while the last head's close-outs drain and the PE
    never idles long enough for a HAM rethrottle at the seam.
  * Phase-3 partials are written f16 (summed in f32 on host); wp loads
    are drip-fed on the idle sync queue during phase 2.
"""

import math
import os
import sys
from collections import deque

import numpy as np

for _p in ("/opt/trn_rl_repo",):
    if _p not in sys.path:
        sys.path.insert(0, _p)

import concourse.mybir as mybir
from concourse import bacc
from concourse.tile import TileContext

B, T, C, H, D = 4, 2048, 2048, 16, 128
P = 128
NCORES = 8
HL = 8          # heads per core (local)
FL = HL * D     # local feature dim = 1024
NCC = C // P    # 16 contraction chunks
NTB = T // 512  # 4 query superblocks
NTC = T // P    # 16 t chunks
EXP_SCALE = 1.0 / math.sqrt(D)
NEG = -1.0e30

f32 = mybir.dt.float32
f16 = mybir.dt.float16


def build_nc():
    nc = bacc.Bacc()
    xt_d = nc.declare_dram_parameter("xt", [C, T], f16, isOutput=False)
    # wqk packed host-side as [jc, p, cc, col]: jc 0..7 = q head jc,
    # jc 8..15 = k head jc-8; one contiguous [128, 16*128] DMA per jc.
    wqk_d = nc.declare_dram_parameter("wqk", [16, P, NCC, P], f16, isOutput=False)
    wv_d = nc.declare_dram_parameter("wv", [NCC, P, FL], f16, isOutput=False)
    wp_d = nc.declare_dram_parameter("wp", [FL, C], f16, isOutput=False)
    masks_d = nc.declare_dram_parameter("masks", [P, P], f32, isOutput=False)
    ones_d = nc.declare_dram_parameter("ones", [P, P], f16, isOutput=False)
    out_d = nc.declare_dram_parameter("out", [T, C], f16, isOutput=True)

    ACT = mybir.ActivationFunctionType

    with TileContext(nc) as tc:
        with tc.tile_pool(name="const", bufs=1) as const_pool, \
             tc.tile_pool(name="qkt", bufs=1) as qkt_pool, \
             tc.tile_pool(name="vv", bufs=1) as v_pool:
            mask_sb = const_pool.tile([P, P], f32)
            nc.gpsimd.dma_start(mask_sb[:], masks_d[:])
            ones_sb = const_pool.tile([P, P], f16)
            nc.gpsimd.dma_start(ones_sb[:], ones_d[:])
            # dummy broadcast: loads the gpsimd Q7 library during phase 1 so
            # the first real phase-2 broadcast doesn't eat the ~6us load
            scratch_bc = const_pool.tile([P, 8], f32)
            nc.gpsimd.partition_broadcast(scratch_bc[:], mask_sb[0:1, 0:8])

            qkts = [qkt_pool.tile([P, T], f16, tag=f"qkt{j}", name=f"qkt{j}")
                    for j in range(16)]
            v_all = v_pool.tile([P, NTC, FL], f16, tag="vall")

            # ---------------- Phase 1: QKV projection ----------------
            with tc.tile_pool(name="xtp", bufs=1) as xt_pool:
                xts = [xt_pool.tile([P, T], f16, tag=f"xt{cc}", name=f"xt{cc}")
                       for cc in range(NCC)]

                with tc.tile_pool(name="ps1", bufs=8, space="PSUM") as ps1, \
                     tc.tile_pool(name="wv1", bufs=1) as wv_pool:
                    wvts = []
                    with tc.tile_pool(name="w1", bufs=3) as w_pool:
                        # q0/k0 weight chunks first so head 0's attention
                        # inputs exist as early as possible (jc 0 = q head 0,
                        # jc 8 = k head 0).
                        w01 = []
                        for jc in (0, 8):
                            wt_ = w_pool.tile([P, NCC, P], f16, tag="wqk",
                                              name=f"wqk{jc}")
                            w01.append(wt_)
                        # scalar queue interleave: tiny first chunks (cc0 of
                        # q0/k0 + first 512 xt cols) so the very first matmul
                        # can fire ~2.5us earlier, then the rest in rate order
                        # so both input streams flow from t=0 and the PE's
                        # chunk consumption is never starved
                        # everything the first ~15us of matmuls consume goes
                        # on the scalar queue in exact first-use order (the
                        # DMA engines drain it preferentially; a sync-queue
                        # chunk was observed arriving 11us late)
                        nc.scalar.dma_start(w01[0][:, 0:1, :], wqk_d[0, :, 0:1, :])
                        nc.scalar.dma_start(xts[0][:, 0:512], xt_d[0:P, 0:512])
                        nc.scalar.dma_start(w01[1][:, 0:1, :], wqk_d[8, :, 0:1, :])
                        nc.scalar.dma_start(xts[0][:, 512:1024],
                                            xt_d[0:P, 512:1024])
                        nc.scalar.dma_start(w01[0][:, 1:4, :], wqk_d[0, :, 1:4, :])
                        nc.scalar.dma_start(xts[0][:, 1024:1536],
                                            xt_d[0:P, 1024:1536])
                        nc.scalar.dma_start(w01[1][:, 1:4, :], wqk_d[8, :, 1:4, :])
                        nc.scalar.dma_start(xts[0][:, 1536:T],
                                            xt_d[0:P, 1536:T])
                        nc.scalar.dma_start(xts[1][:], xt_d[P:2 * P, :])
                        nc.sync.dma_start(xts[2][:], xt_d[2 * P:3 * P, :])
                        nc.scalar.dma_start(xts[3][:], xt_d[3 * P:4 * P, :])
                        nc.scalar.dma_start(w01[0][:, 4:NCC, :],
                                            wqk_d[0, :, 4:NCC, :])
                        nc.scalar.dma_start(w01[1][:, 4:NCC, :],
                                            wqk_d[8, :, 4:NCC, :])
                        for cc in range(4, NCC):
                            eng = nc.sync if cc % 2 == 0 else nc.scalar
                            eng.dma_start(
                                xts[cc][:], xt_d[cc * P:(cc + 1) * P, :])
                        # PE warm-up: the HAM clock gate only reaches 8/8
                        # after ~3.4us of sustained matmul activity, and the
                        # first real matmuls are DMA-starved until ~14us, so
                        # without this the whole first ~10us of real work runs
                        # at 1.2 GHz. Dummy matmuls on a memset tile (no DMA
                        # dependency) fill the DMA-wait dead time instead.
                        warm_sb = const_pool.tile([P, 512], f16,
                                                  name="warm_sb")
                        nc.vector.memset(warm_sb[:], 0)
                        warm_ps = ps1.tile([P, 512], f32, tag="ps1",
                                           name="warm_ps")
                        for wu in range(24):
                            nc.tensor.matmul(
                                warm_ps[:], warm_sb[:, 0:P], warm_sb[:],
                                start=(wu == 0), stop=(wu == 23))

                        # jc 0/8 emitted cc-outer: the PE's in-order queue
                        # then tracks xt chunk arrivals instead of blocking
                        # on the full xt load.
                        ps01 = [ps1.tile([P, 512], f32, tag="ps1",
                                         name=f"ps01_{g}") for g in range(8)]
                        for cc in range(NCC):
                            for wi in range(2):
                                for tb in range(NTB):
                                    nc.tensor.matmul(
                                        ps01[wi * NTB + tb][:],
                                        w01[wi][:, cc, :],
                                        xts[cc][:, tb * 512:(tb + 1) * 512],
                                        start=(cc == 0), stop=(cc == NCC - 1))
                        for wi, jc in enumerate((0, 8)):
                            for tb in range(NTB):
                                nc.scalar.copy(
                                    out=qkts[jc][:, tb * 512:(tb + 1) * 512],
                                    in_=ps01[wi * NTB + tb][:])

                        for jc in [j for j in range(16) if j not in (0, 8)]:
                            wt_ = w_pool.tile([P, NCC, P], f16, tag="wqk")
                            nc.scalar.dma_start(wt_[:], wqk_d[jc])
                            # cc-outer / tb-inner: 4 consecutive matmuls share
                            # the same stationary tile
                            pss = [ps1.tile([P, 512], f32, tag="ps1",
                                            name=f"psj{jc}_{tb}")
                                   for tb in range(NTB)]
                            for cc in range(NCC):
                                for tb in range(NTB):
                                    nc.tensor.matmul(
                                        pss[tb][:], wt_[:, cc, :],
                                        xts[cc][:, tb * 512:(tb + 1) * 512],
                                        start=(cc == 0), stop=(cc == NCC - 1))
                            for tb in range(NTB):
                                nc.scalar.copy(
                                    out=qkts[jc][:, tb * 512:(tb + 1) * 512],
                                    in_=pss[tb][:])

                        # wv loads stream behind the wqk stream during P1a so
                        # phase 1b starts without a handoff stall
                        for cc in range(NCC):
                            wt_ = wv_pool.tile([P, FL], f16, tag=f"wv{cc}",
                                               name=f"wv{cc}")
                            nc.scalar.dma_start(wt_[:], wv_d[cc])
                            wvts.append(wt_)

                    # 1b: v in natural [t, d] layout
                    if True:
                        for vb in range(FL // 512):
                            for tch in range(NTC):
                                ps = ps1.tile([P, 512], f32, tag="ps1")
                                for cc in range(NCC):
                                    nc.tensor.matmul(
                                        ps[:],
                                        xts[cc][:, tch * P:(tch + 1) * P],
                                        wvts[cc][:, vb * 512:(vb + 1) * 512],
                                        start=(cc == 0), stop=(cc == NCC - 1))
                                nc.vector.tensor_copy(
                                    out=v_all[:, tch, vb * 512:(vb + 1) * 512],
                                    in_=ps[:])

            # ---------------- Phase 2: attention ----------------
            with tc.tile_pool(name="avt", bufs=1) as avt_pool, \
                 tc.tile_pool(name="wpp", bufs=1) as wp_pool:
                avts = [avt_pool.tile([P, T], f16, tag=f"avt{h}", name=f"avt{h}")
                        for h in range(HL)]
                wps = [wp_pool.tile([P, C], f16, tag=f"wp{f}", name=f"wp{f}")
                       for f in range(HL)]

                with tc.tile_pool(name="p2sb", bufs=1) as p2sb, \
                     tc.tile_pool(name="ps2", bufs=1, space="PSUM") as ps2:

                    class Grp:
                        __slots__ = ("h", "si", "njc", "qt", "kt", "av_ps",
                                     "sumacc", "recb", "last_et", "last_infos",
                                     "first_et", "first_infos")

                    def si_group(h, si, qt, kt):
                        g = Grp()
                        g.h, g.si, g.njc, g.qt, g.kt = h, si, 4 * si + 4, qt, kt
                        g.av_ps = ps2.tile([P, 512], f32, tag="av",
                                           bufs=2, name=f"av_{h}_{si}")
                        # [key-part, chunk-parity x query]: clean pairs fold in
                        # with ONE full-width DVE add; the den ones-matmul sums
                        # both halves (PE has slack, DVE is the bottleneck).
                        # si 0 has a single non-last pair: its den reads both
                        # et tiles directly -- no sumacc, no DVE memset/adds.
                        g.sumacc = None if si == 0 else \
                            p2sb.tile([P, 1024], f16, tag="sum", bufs=3)
                        g.recb = None
                        return [(g, pp) for pp in range(g.njc // 2)]

                    def weave(a, b, frac=0.55):
                        """Spread the short group b inside the FIRST `frac` of
                        the long group a: the pipeline never runs shallow,
                        diagonal (masked, partial-width) pairs are kept apart,
                        and the two groups' close-out chains (den/recip/
                        broadcast) finish >=3 pairs apart so they never
                        serialize on gpsimd at a block boundary."""
                        out, bi = [], 0
                        na, nb = len(a), len(b)
                        for i, r in enumerate(a):
                            out.append(r)
                            while bi < nb and (bi + 1) * na * frac <= \
                                    (i + 1) * nb:
                                out.append(b[bi])
                                bi += 1
                        out.extend(b[bi:])
                        return out

                    # wp loads drip-fed on the (otherwise idle in phase 2)
                    # sync queue after the phase boundary; one per AV pair
                    # during head 1 so no single burst delays anything
                    wpq = [f for f in range(HL)]

                    def gen_pairs():
                        """Yield per-pair emission records across all
                        (head, si); scores run LOOK pairs ahead of AV."""
                        for h in range(HL):
                            qt, kt = qkts[h], qkts[8 + h]
                            # group CREATION order (3,1,0,2) sets the av_ps
                            # bank rotation: si0 inherits si3's bank (first
                            # reused 3 pairs into block 2) and si2 inherits
                            # si1's long-released bank, so no first-AV ever
                            # waits on a close-out multiply that just fired
                            # (the emission order itself is unchanged).
                            a = si_group(h, 3, qt, kt)
                            b = si_group(h, 1, qt, kt)
                            e = si_group(h, 0, qt, kt)
                            c = si_group(h, 2, qt, kt)
                            recs = weave(a, b)
                            recs += weave(c, e, frac=0.5)
                            yield from recs

                    def emit_sc(rec):
                        g, pp = rec
                        si = g.si
                        sc = ps2.tile([P, 1024], f32, tag="sc", bufs=3)
                        et = p2sb.tile([P, 1024], f16, tag="et", bufs=10)
                        infos = []
                        for jj in (0, 1):
                            j = 2 * pp + jj
                            diag = j * P - si * 512
                            d_off = max(0, diag)
                            # left-align the second diagonal block so the
                            # pair's valid region is contiguous -> one exp
                            base = jj * 512 - (d_off if jj else 0)
                            nc.tensor.matmul(
                                sc[:, base + d_off:base + 512],
                                g.kt[:, j * P:(j + 1) * P],
                                g.qt[:, si * 512 + d_off:(si + 1) * 512],
                                start=True, stop=True)
                            infos.append((j, diag >= 0, d_off, base))
                        # masks immediately after the score matmuls so exp is
                        # never queued behind lower-priority DVE work. A pair
                        # whose blocks are BOTH diagonal merges its two mask
                        # adds into ONE strided DVE op: with left-aligned
                        # bases the regions always sit at d_off0 and 512 with
                        # equal 128 widths (halves the mask-chain latency in
                        # front of the exp).
                        if infos[0][1] and infos[1][1]:
                            d0 = infos[0][2]
                            stw = 512 - d0
                            reg = sc[:, d0:d0 + 2 * stw].rearrange(
                                "p (b w) -> p b w", b=2)[:, :, 0:P]
                            mb = mask_sb[:].unsqueeze(1).broadcast_to(
                                (P, 2, P))
                            nc.vector.tensor_add(out=reg, in0=reg, in1=mb)
                        else:
                            for (j, is_diag, d_off, base) in infos:
                                if is_diag:
                                    nc.vector.tensor_add(
                                        out=sc[:, base + d_off:
                                               base + d_off + P],
                                        in0=sc[:, base + d_off:
                                               base + d_off + P],
                                        in1=mask_sb[:])
                        lo = infos[0][2] + infos[0][3]  # d_off0 (+ base0 == 0)
                        hi = infos[1][3] + 512          # base1 + 512
                        nc.scalar.activation(et[:, lo:hi], sc[:, lo:hi],
                                             ACT.Exp, scale=EXP_SCALE)
                        # flush the previous pair's lazily-queued sums now
                        # that this pair's masks+exp are already in the FIFOs
                        while sum_q:
                            sum_q.popleft()()
                        last_pair = (2 * pp + 1 == g.njc - 1)
                        if not last_pair:
                            # denominator accumulation trails the exps; it
                            # only gates the si-end ones-matmuls, never the
                            # next exp. Sums are emitted LAZILY (one pair
                            # later, via sum_q) so the next pair's exp-gating
                            # mask adds never queue behind them in the DVE
                            # FIFO. First (clean) pair initializes sumacc
                            # with a 4x-mode copy; later clean pairs fold in
                            # with one full-width 2x add; diagonal pairs add
                            # their two left-aligned valid regions into the
                            # matching sumacc halves.
                            clean = not (infos[0][1] or infos[1][1])
                            if si == 0:
                                g.first_et, g.first_infos = et, infos
                            elif pp == 0:
                                sum_q.append(lambda g=g, et=et:
                                             nc.vector.tensor_copy(
                                                 out=g.sumacc[:], in_=et[:]))
                            elif clean:
                                sum_q.append(lambda g=g, et=et:
                                             nc.vector.tensor_add(
                                                 out=g.sumacc[:],
                                                 in0=g.sumacc[:],
                                                 in1=et[:]))
                            else:
                                def diag_sums(g=g, et=et, infos=infos):
                                    for (j, is_diag, d_off, base) in infos:
                                        half = (j % 2) * 512
                                        nc.vector.tensor_add(
                                            out=g.sumacc[:, half + d_off:
                                                         half + 512],
                                            in0=g.sumacc[:, half + d_off:
                                                         half + 512],
                                            in1=et[:, base + d_off:
                                                   base + 512])
                                sum_q.append(diag_sums)
                        else:
                            g.last_et, g.last_infos = et, infos
                        return (rec, et, infos, last_pair)

                    def emit_den(g):
                        # Softmax close-out, emitted DEN_DELAY pairs after the
                        # group's last score pair -- i.e. *well before* the
                        # group's last AV matmuls (which trail by LOOK pairs).
                        # It depends only on exps/sumacc, so the reciprocal
                        # and its gpsimd broadcast overlap the AV stream and
                        # the final rescale multiply never stalls the PE.
                        # The last pair's exps are summed directly by PE
                        # ones-matmuls (PSUM accumulation) to spare DVE.
                        # 1-column ones stationary: a wider (broadcasting)
                        # ones matmul trips the P0 power downclock (PE drops
                        # 2.4->2.0 GHz for the whole kernel). The broadcast
                        # rides gpsimd, whose Q7 library was preloaded in
                        # phase 1 so no load stall lands here.
                        den = ps2.tile([P, 1024], f32, tag="sc", bufs=3,
                                       name=f"den_{g.h}_{g.si}")
                        if g.si == 0:
                            # no sumacc: read both pairs' ets directly. The
                            # first region (j0, d_off=0) is full width, so it
                            # anchors the accumulation group.
                            regions = [(g.first_et, inf) for inf in
                                       g.first_infos] + \
                                      [(g.last_et, inf) for inf in
                                       g.last_infos]
                            for ri, (et_, (j, is_diag, d_off, base)) in \
                                    enumerate(regions):
                                nc.tensor.matmul(
                                    den[0:1, d_off:512], ones_sb[:, 0:1],
                                    et_[:, base + d_off:base + 512],
                                    start=(ri == 0), stop=(j == g.njc - 1))
                        else:
                            nc.tensor.matmul(
                                den[0:1, 0:512], ones_sb[:, 0:1],
                                g.sumacc[:, 0:512], start=True, stop=False)
                            nc.tensor.matmul(
                                den[0:1, 0:512], ones_sb[:, 0:1],
                                g.sumacc[:, 512:1024], start=False, stop=False)
                            for (j, is_diag, d_off, base) in g.last_infos:
                                nc.tensor.matmul(
                                    den[0:1, d_off:512], ones_sb[:, 0:1],
                                    g.last_et[:, base + d_off:base + 512],
                                    start=False, stop=(j == g.njc - 1))
                        rec32 = p2sb.tile([1, 512], f32, tag="rec32",
                                          bufs=3)
                        nc.vector.reciprocal_approx_fast(
                            out=rec32[:], in_=den[0:1, 0:512])
                        recb = p2sb.tile([P, 512], f32, tag="recb",
                                         bufs=3)
                        nc.gpsimd.partition_broadcast(recb[:], rec32[:])
                        g.recb = recb

                    def emit_av(sc_rec):
                        (g, pp), et, infos, last_pair = sc_rec
                        h, si = g.h, g.si
                        if h >= 1 and wpq:
                            f = wpq.pop(0)
                            nc.sync.dma_start(
                                wps[f][:], wp_d[f * P:(f + 1) * P, :])
                        for (j, is_diag, d_off, base) in infos:
                            nc.tensor.matmul(
                                g.av_ps[:, d_off:],
                                v_all[:, j, h * P:(h + 1) * P],
                                et[:, base + d_off:base + 512],
                                start=(j == 0), stop=(j == g.njc - 1))
                        if last_pair:
                            if g.recb is None:
                                emit_den(g)
                            nc.vector.tensor_mul(
                                out=avts[h][:, si * 512:(si + 1) * 512],
                                in0=g.av_ps[:], in1=g.recb[:])

                    LOOK = 7
                    DEN_DELAY = 4  # effective delay: DEN_DELAY-1 sc pairs
                    pend = deque()
                    denq = deque()  # (group, countdown to den emission)
                    sum_q = deque()  # lazily-emitted sum thunks
                    for rec in gen_pairs():
                        sc_rec = emit_sc(rec)
                        pend.append(sc_rec)
                        if sc_rec[3]:
                            denq.append([sc_rec[0][0], DEN_DELAY])
                        for e in denq:
                            e[1] -= 1
                        while denq and denq[0][1] <= 0:
                            emit_den(denq.popleft()[0])
                        if len(pend) > LOOK:
                            emit_av(pend.popleft())
                    while sum_q:
                        sum_q.popleft()()
                    while denq:
                        emit_den(denq.popleft()[0])
                    while pend:
                        emit_av(pend.popleft())

                    # ------------ Phase 3: output projection ------------
                    # Merged into the phase-2 pool scope (PSUM tiles reuse
                    # tag "sc") so there is no pool-transition stall and no
                    # PE idle window for a HAM rethrottle. tch order follows
                    # avts readiness (head 7 closes si1, si3, si0, si2) so
                    # the first projection matmuls only depend on muls that
                    # fired pairs ago, and the late close-outs drain while
                    # the PE is already projecting.
                    p3_order = [4, 5, 6, 7, 12, 13, 14, 15,
                                0, 1, 2, 3, 8, 9, 10, 11]
                    for oi, tch in enumerate(p3_order):
                        ob = p2sb.tile([P, C], f16, tag="ocb", bufs=2)
                        for hb in range(2):
                            ps = ps2.tile([P, 1024], f32, tag="sc", bufs=3,
                                          name=f"p3_{tch}_{hb}")
                            for half in range(2):
                                cb_ = hb * 2 + half
                                for f in range(HL):
                                    nc.tensor.matmul(
                                        ps[:, half * 512:(half + 1) * 512],
                                        avts[f][:, tch * P:(tch + 1) * P],
                                        wps[f][:, cb_ * 512:(cb_ + 1) * 512],
                                        start=(f == 0), stop=(f == HL - 1))
                            # one wide copy per [P,1024] accumulation tile
                            if hb == 0:
                                nc.scalar.copy(
                                    out=ob[:, 0:1024], in_=ps[:])
                            else:
                                nc.vector.tensor_copy(
                                    out=ob[:, 1024:2048], in_=ps[:])
                        if oi >= NTC - 2:
                            # split the final transfers so the drain tail is
                            # one small DMA, not a 1MB one
                            for cb_ in range(C // 512):
                                eng = nc.sync if cb_ % 2 == 0 else nc.scalar
                                eng.dma_start(
                                    out_d[tch * P:(tch + 1) * P,
                                          cb_ * 512:(cb_ + 1) * 512],
                                    ob[:, cb_ * 512:(cb_ + 1) * 512])
                        else:
                            eng = nc.sync if oi % 2 == 0 else nc.scalar
                            eng.dma_start(
                                out_d[tch * P:(tch + 1) * P, :], ob[:])
    nc.compile()
    return nc


def _make_masks():
    pp = np.arange(P)[:, None]
    ff = np.arange(P)[None, :]
    return np.where(ff >= pp, 0.0, NEG).astype(np.float32)


def _prep_inputs(x, w_qkv, w_proj):
    masks = _make_masks()
    per_g = {}
    for g in range(2):
        cols = slice(g * FL, (g + 1) * FL)
        wqk_c = np.concatenate(
            [w_qkv[:, :C][:, cols], w_qkv[:, C:2 * C][:, cols]], axis=1)
        # [jc, p, cc, col]: one contiguous DMA per jc
        wqk_packed = np.ascontiguousarray(
            wqk_c.reshape(NCC, P, 16, P).transpose(2, 1, 0, 3)).astype(np.float16)
        wv_c = np.ascontiguousarray(
            w_qkv[:, 2 * C:][:, cols].reshape(NCC, P, FL)).astype(np.float16)
        wp_c = np.ascontiguousarray(
            w_proj[g * FL:(g + 1) * FL, :]).astype(np.float16)
        per_g[g] = (wqk_packed, wv_c, wp_c)
    in_maps = []
    for core in range(NCORES):
        b, g = core // 2, core % 2
        wqk_packed, wv_c, wp_c = per_g[g]
        in_maps.append({
            "xt": np.ascontiguousarray(x[b].T).astype(np.float16),
            "wqk": wqk_packed,
            "wv": wv_c,
            "wp": wp_c,
            "masks": masks,
            "ones": np.ones((P, P), dtype=np.float16),
        })
    return in_maps


_nc_cache = None
last_results = None  # BassKernelResults of the most recent run (for test.py)


def kernel(x, w_qkv, w_proj):
    global _nc_cache, last_results
    from concourse.bass_utils import run_bass_kernel_spmd

    x = np.asarray(x, dtype=np.float32)
    w_qkv = np.asarray(w_qkv, dtype=np.float32)
    w_proj = np.asarray(w_proj, dtype=np.float32)

    if _nc_cache is None:
        _nc_cache = build_nc()
    nc = _nc_cache

    in_maps = _prep_inputs(x, w_qkv, w_proj)
    trace = bool(int(os.environ.get("KERNEL_TRACE", "0")))
    res = run_bass_kernel_spmd(nc, in_maps, list(range(NCORES)), trace=trace)
    last_results = res

    out = np.empty((B, T, C), dtype=np.float32)
    for b in range(B):
        out[b] = (res.results[2 * b]["out"].astype(np.float32)
                  + res.results[2 * b + 1]["out"].astype(np.float32))
    return out

